# revision 21
# baseline (speedup 1.0000x reference)
"""Multi-headed self-attention (S=2048, D=1024, H=16) on 8 trn2 NeuronCores.

Sharding: tensor-parallel over heads (2 heads/core), fully collective-free.
Each core computes q/k/v for its 2 heads, runs base-2 no-max softmax
attention, and then computes the PARTIAL output projection
ctx_local.T @ w_out_local.T for the full [S, D] output (the projection is
K-split over heads). The host gather/unshard step sums the 8 partial
outputs. No cross-core communication means no collective latency and no
sensitivity to core launch skew.

Pipeline notes (all measured on HW traces):
- 6 batched input DMAs; x split into 4 tiles so qkv matmuls start as
  d-tile pairs arrive (DMA deps are tracked per tile write).
- Every matmul uses the same (128,128)@(0,0) PE tile config (scores use
  per-head zero-padded k.T) -- mixed configs pin the PE at 1.2 GHz.
- Attention inner loop is software-pipelined: scores(tt+1) is emitted
  before ctx(tt) so the in-order PE queue never stalls on the Exp.
- Softmax denominator from a fused ones-column in v'; normalization via
  reciprocal + gpsimd partition_broadcast + one DVE multiply.
- proj(chunk 0) is emitted in the middle of chunk 1's attention so its
  normalize dependency is long satisfied; only chunk 1's normalize+proj
  are in the tail.

Self-contained: hardcodes all shapes; host-side prep is limited to
transpose / dtype-cast / slicing / concatenation of the inputs, and the
unshard step sums the per-core partial outputs.
"""

import sys

import numpy as np

if "/opt/trn_rl_repo" not in sys.path:
    sys.path.insert(0, "/opt/trn_rl_repo")

S, D, A, H = 2048, 1024, 1024, 16
NCORES = 8
HPC = H // NCORES            # heads per core = 2
HD = A // H                  # head dim = 64
E = HPC * HD                 # local qkv rows = 128
ND = D // 128                # d tiles = 8
NT = S // 128                # t tiles = 16
LN2 = 0.6931471805599453
EXP_SCALE = LN2 * (HD ** -0.5)   # p = 2^(score/8) = exp(score * ln2/8)

NCH = 2                      # attention s-chunks
CH = S // NCH                # 1024
NB = CH // 128               # proj s-blocks per chunk = 8

_CACHE = {}


def _build(enable_asserts=False):
    import concourse.bass as bass
    import concourse.tile as tile
    import concourse.mybir as mybir
    from concourse import bacc
    from concourse.masks import make_identity

    f16 = mybir.dt.float16

    nc = bacc.Bacc(
        "TRN2",
        target_bir_lowering=False,
        debug=False,
        enable_asserts=enable_asserts,
        num_devices=NCORES,
    )

    # xT: x.T as [ND, 128, S] (d-tile major); wqkv: [ND, 128, 3E] packed
    # q|k|v columns; wol: this core's w_out.T row strip [128, D].
    xT = nc.dram_tensor("xT", [ND, 128, S], f16, kind="ExternalInput").ap()
    wqkv = nc.dram_tensor("wqkv", [ND, 128, 3 * E], f16, kind="ExternalInput").ap()
    wol = nc.dram_tensor("wol", [128, D], f16, kind="ExternalInput").ap()
    # partial output, s-block major: rows s = ci*CH + b*128 + p
    out = nc.dram_tensor("out", [NCH, NB, 128, D], f16, kind="ExternalOutput").ap()

    with tile.TileContext(nc) as tc:
        _body(tc, xT, wqkv, wol, out, mybir, bass, make_identity)

    nc.compile()
    return nc


def _body(tc, xT, wqkv, wol, out, mybir, bass, make_identity):
    from contextlib import ExitStack

    nc = tc.nc
    f16 = mybir.dt.float16
    f32 = mybir.dt.float32
    Exp = mybir.ActivationFunctionType.Exp

    ctx_stack = ExitStack()
    persist = ctx_stack.enter_context(tc.tile_pool(name="persist", bufs=1))

    def ptile(shape, dtype, name):
        return persist.tile(shape, dtype, tag=name, name=name)

    # x.T d-tile major, split into one tile per load DMA for exact deps
    xt_g = [ptile([128, S], f16, f"xt_g{g}") for g in range(ND)]
    wqkv_sb = ptile([128, ND, 3 * E], f16, "wqkv_sb")
    wol_sb = ptile([128, D], f16, "wol_sb")
    # q.T split per attention chunk (exact deps on the q-pass copies)
    qT_c = [ptile([128, CH], f16, f"qT_c{ci}") for ci in range(NCH)]
    # per-head k.T zero-padded to K=128 so the scores matmul uses the same
    # (128,128) PE tile config as every other matmul
    kT2_sb = [ptile([128, S], f16, f"kT2_sb{h}") for h in range(HPC)]
    vT_sb = ptile([128, S], f16, "vT_sb")
    # v' per t-tile: [v_h0 | ones | v_h1 | ones] -> lhsT cols [0:65], [65:130]
    vp_sb = ptile([128, NT, 2 * (HD + 1)], f16, "vp_sb")
    ident_sb = ptile([128, 128], f16, "ident_sb")
    # normalized ctx.T, both heads: rows [h*64:(h+1)*64], cols = s
    ctxn_sb = ptile([128, S], f16, "ctxn_sb")
    # junk operand for PE clock-warmup matmuls (contents irrelevant)
    junk_sb = ptile([128, 512], f16, "junk_sb")

    make_identity(nc, ident_sb[:])

    # ---- batched input loads; the two HWDGE queues (sync, scalar)
    # drain in issue order and feed the DMA engines in parallel ----
    nc.sync.dma_start(wqkv_sb[:], wqkv.rearrange("t p c -> p t c"))
    nc.scalar.dma_start(wol_sb[:], wol)
    for g in range(ND):
        eng = nc.sync if g % 2 == 0 else nc.scalar
        eng.dma_start(xt_g[g][:], xT[g])

    # zero the pad halves of the per-head k tensors once, before the k-pass
    nc.vector.memset(kT2_sb[0][HD:128, :], 0.0)
    nc.vector.memset(kT2_sb[1][0:HD, :], 0.0)

    # PE clock warmup: dummy matmuls with no data deps run while the input
    # DMAs are still in flight, ramping the PE to the fast p-state before
    # the first qkv matmul
    nc.vector.memset(junk_sb[:], 0.0)
    with tc.tile_pool(name="warm_ps", bufs=1, space="PSUM") as warm_ps:
        wp = warm_ps.tile([128, 512], f32, tag="wp", name="warm")
        for i in range(16):
            nc.tensor.matmul(
                wp[:], lhsT=junk_sb[:, 0:128], rhs=junk_sb[:],
                start=True, stop=True,
            )

    # ---- qkv.T = w.T^T @ x.T : d-tile outer so each weight LDW feeds 4 MMs
    # pass order k, q, v (scores need k/q first). PSUM->SBUF copies on the
    # scalar engine (idle until the first Exp).
    with tc.tile_pool(name="qkv_ps", bufs=2, space="PSUM") as qkv_ps:
        for w_off, dst in ((E, None), (0, qT_c), (2 * E, vT_sb)):
            pss = [
                qkv_ps.tile([128, 512], f32, tag=f"qkv{i}", name=f"qkv{i}")
                for i in range(4)
            ]
            for dt_ in range(ND):
                for sc4 in range(4):
                    nc.tensor.matmul(
                        pss[sc4][:],
                        lhsT=wqkv_sb[:, dt_, w_off:w_off + E],
                        rhs=xt_g[dt_][:, sc4 * 512:(sc4 + 1) * 512],
                        start=(dt_ == 0),
                        stop=(dt_ == ND - 1),
                    )
            for sc4 in range(4):
                cols = slice(sc4 * 512, (sc4 + 1) * 512)
                if dst is None:      # k: split per head into zero-padded kT2
                    nc.scalar.copy(kT2_sb[0][0:HD, cols], pss[sc4][0:HD, :])
                    nc.scalar.copy(kT2_sb[1][HD:128, cols], pss[sc4][HD:128, :])
                elif isinstance(dst, list):   # q: per-chunk tiles
                    nc.scalar.copy(
                        dst[sc4 // 2][:, (sc4 % 2) * 512:(sc4 % 2 + 1) * 512],
                        pss[sc4][:],
                    )
                else:
                    nc.scalar.copy(dst[:, cols], pss[sc4][:])

    # ---- v' = v.T transposed back per t-tile, plus ones columns ----
    with tc.tile_pool(name="tr_ps", bufs=3, space="PSUM") as tr_ps:
        for tt in range(NT):
            tp = tr_ps.tile([128, 128], f16, tag="tr")
            nc.tensor.transpose(
                tp[:], vT_sb[:, tt * 128:(tt + 1) * 128], ident_sb[:]
            )
            nc.vector.tensor_copy(vp_sb[:, tt, 0:HD], tp[:, 0:HD])
            nc.vector.tensor_copy(
                vp_sb[:, tt, HD + 1:2 * HD + 1], tp[:, HD:2 * HD]
            )
        nc.vector.memset(vp_sb[:, :, HD:HD + 1], 1.0)
        nc.vector.memset(vp_sb[:, :, 2 * HD + 1:2 * HD + 2], 1.0)

    # ---- attention (chunk outer) + per-chunk partial projection ----
    with (
        tc.tile_pool(name="sc_ps", bufs=2, space="PSUM") as sc_ps,
        tc.tile_pool(name="ctx_ps", bufs=2, space="PSUM") as ctx_ps,
        tc.tile_pool(name="pt_pool", bufs=4) as pt_pool,
        tc.tile_pool(name="nrm_pool", bufs=2) as nrm_pool,
        tc.tile_pool(name="out_pool", bufs=2) as out_pool,
    ):
        def attn(ci, h, interleave=None):
            hb = h * HD
            ctx = ctx_ps.tile([HD + 1, CH], f32, tag="ctx", name="ctx")

            def ctx_mm(tt, pt):
                for nn in range(CH // 512):
                    nc.tensor.matmul(
                        ctx[:, nn * 512:(nn + 1) * 512],
                        lhsT=vp_sb[:, tt, h * (HD + 1):(h + 1) * (HD + 1)],
                        rhs=pt[:, nn * 512:(nn + 1) * 512],
                        start=(tt == 0),
                        stop=(tt == NT - 1),
                    )

            # software-pipelined: emit scores(tt) before ctx(tt-1) so the
            # in-order PE queue never stalls on exp(tt)
            pts = {}
            for tt in range(NT):
                sc = sc_ps.tile([128, CH], f32, tag="sc", name="sc")
                for nn in range(CH // 512):
                    nc.tensor.matmul(
                        sc[:, nn * 512:(nn + 1) * 512],
                        lhsT=kT2_sb[h][:, tt * 128:(tt + 1) * 128],
                        rhs=qT_c[ci][:, nn * 512:(nn + 1) * 512],
                        start=True,
                        stop=True,
                    )
                pt = pt_pool.tile([128, CH], f16, tag="pt")
                nc.scalar.activation(pt[:], sc[:], Exp, scale=EXP_SCALE)
                pts[tt] = pt
                if tt >= 1:
                    ctx_mm(tt - 1, pts.pop(tt - 1))
                if interleave is not None and tt in interleave:
                    interleave[tt]()
            ctx_mm(NT - 1, pts.pop(NT - 1))

            # normalize: copy+recip of the denominator row (the custom
            # DVE recip op cannot read PSUM), partition-broadcast, DVE
            # multiply into ctxn rows [hb:hb+64]. The last block runs in
            # column halves so the tail projection starts ~3us earlier.
            nhalves = 2 if (ci, h) == (NCH - 1, HPC - 1) else 1
            W = CH // nhalves
            for hf in range(nhalves):
                cs = slice(hf * W, (hf + 1) * W)
                den = nrm_pool.tile([1, W], f32, tag=f"den{nhalves}",
                                    name="den")
                nc.vector.tensor_copy(den[:], ctx[HD:HD + 1, cs])
                rec = nrm_pool.tile([1, W], f32, tag=f"rec{nhalves}",
                                    name="rec")
                nc.vector.reciprocal_approx_fast(rec[:], den[:])
                rb = nrm_pool.tile([HD, W], f32, tag=f"rb{nhalves}",
                                   name="rb")
                nc.gpsimd.partition_broadcast(rb[:], rec[:])
                nc.vector.tensor_mul(
                    ctxn_sb[hb:hb + HD,
                            ci * CH + hf * W:ci * CH + (hf + 1) * W],
                    ctx[0:HD, cs],
                    rb[:],
                )

        pair_obs = {}

        def proj_block(ci, b):
            # partial out rows [ci*CH + b*128 : +128]:
            # out[s, :] += ctxn[:, s].T @ wol  (K = this core's 128 A-rows)
            if b % 2 == 0:
                pair_obs[ci] = out_pool.tile([128, 2, D], f16, tag="ob",
                                             name="ob")
            ob = pair_obs[ci]
            ps = sc_ps.tile([128, CH], f32, tag="sc", name="proj")
            for nn in range(2):
                nc.tensor.matmul(
                    ps[:, nn * 512:(nn + 1) * 512],
                    lhsT=ctxn_sb[:, ci * CH + b * 128:
                                 ci * CH + (b + 1) * 128],
                    rhs=wol_sb[:, nn * 512:(nn + 1) * 512],
                    start=True,
                    stop=True,
                )
            nc.vector.tensor_copy(ob[:, b % 2, :], ps[:])
            if b % 2 == 1:   # ship each 2-block pair as soon as it's cast
                nc.sync.dma_start(
                    out[ci, b - 1:b + 1].rearrange("b p d -> p b d"), ob[:]
                )

        attn(0, 0)
        attn(0, 1)
        # proj(0) blocks are interleaved into attn(1,0)'s tt loop: the
        # scalar engine keeps running exps while the PE absorbs the
        # projection matmuls in its slack.
        attn(1, 0, interleave={4 + 3 * i: (lambda i=i: proj_block(0, i))
                               for i in range(4)})
        attn(1, 1, interleave={1 + 3 * i: (lambda i=i: proj_block(0, 4 + i))
                               for i in range(4)})

        # dummy matmuls on junk data bridge the normalize(1,1) window and
        # the projection's CAST waits so the PE clock stays at the fast
        # p-state through the tail
        def junk_mm(n):
            for _ in range(n):
                jp = sc_ps.tile([128, CH], f32, tag="sc", name="junk")
                nc.tensor.matmul(
                    jp[:, 0:512], lhsT=junk_sb[:, 0:128],
                    rhs=junk_sb[:, 0:512], start=True, stop=True,
                )

        junk_mm(8)
        for b in range(NB):
            proj_block(1, b)
            junk_mm(2)

    ctx_stack.close()


def get_nc(enable_asserts=False):
    key = ("nc", enable_asserts)
    if key not in _CACHE:
        _CACHE[key] = _build(enable_asserts)
    return _CACHE[key]


def make_in_maps(x, w_in, w_out):
    x = np.asarray(x, dtype=np.float32)
    w_in = np.asarray(w_in, dtype=np.float32)
    w_out = np.asarray(w_out, dtype=np.float32)
    xT = np.ascontiguousarray(x.T).astype(np.float16).reshape(ND, 128, S)
    w_outT = np.ascontiguousarray(w_out.T).astype(np.float16)  # [A, D]
    in_maps = []
    for c in range(NCORES):
        r0 = c * E
        wq = w_in[r0:r0 + E].T
        wk = w_in[A + r0:A + r0 + E].T
        wv = w_in[2 * A + r0:2 * A + r0 + E].T
        wqkv = np.ascontiguousarray(
            np.concatenate([wq, wk, wv], axis=1)
        ).astype(np.float16).reshape(ND, 128, 3 * E)
        wol = np.ascontiguousarray(w_outT[r0:r0 + E])  # [128, D]
        in_maps.append({"xT": xT, "wqkv": wqkv, "wol": wol})
    return in_maps


def assemble_out(results):
    """results[c]["out"] is [NCH, NB, 128, D] fp16 partials in s-block
    order; the unshard step sums the 8 cores' partial projections."""
    full = np.zeros((S, D), dtype=np.float32)
    for c in range(NCORES):
        o = results[c]["out"].astype(np.float32).reshape(S, D)
        full += o
    return full


def kernel(x, w_in, w_out, tgt_len=None, **kwargs):
    from concourse.bass_utils import run_bass_kernel_spmd

    nc = get_nc()
    in_maps = make_in_maps(x, w_in, w_out)
    res = run_bass_kernel_spmd(nc, in_maps, core_ids=list(range(NCORES)))
    return assemble_out(res.results)


# revision 22
# speedup vs baseline: 1.1828x; 1.1828x over previous
"""Multi-headed self-attention (S=2048, D=1024, H=16) on 8 trn2 NeuronCores.

Sharding: tensor-parallel over heads (2 heads/core), fully collective-free.
Each core computes q/k/v for its 2 heads, runs base-2 no-max softmax
attention, and then computes the PARTIAL output projection
ctx_local.T @ w_out_local.T for the full [S, D] output (the projection is
K-split over heads). The host gather/unshard step sums the 8 partial
outputs. No cross-core communication means no collective latency and no
sensitivity to core launch skew.

Pipeline notes (all measured on HW traces):
- 6 batched input DMAs; x split into 4 tiles so qkv matmuls start as
  d-tile pairs arrive (DMA deps are tracked per tile write).
- Every matmul uses the same (128,128)@(0,0) PE tile config (scores use
  per-head zero-padded k.T) -- mixed configs pin the PE at 1.2 GHz.
- Attention inner loop is software-pipelined: scores(tt+1) is emitted
  before ctx(tt) so the in-order PE queue never stalls on the Exp.
- Softmax denominator from a fused ones-column in v'; normalization via
  reciprocal + gpsimd partition_broadcast + one DVE multiply.
- proj(chunk 0) is emitted in the middle of chunk 1's attention so its
  normalize dependency is long satisfied; only chunk 1's normalize+proj
  are in the tail.

Self-contained: hardcodes all shapes; host-side prep is limited to
transpose / dtype-cast / slicing / concatenation of the inputs, and the
unshard step sums the per-core partial outputs.
"""

import sys

import numpy as np

if "/opt/trn_rl_repo" not in sys.path:
    sys.path.insert(0, "/opt/trn_rl_repo")

S, D, A, H = 2048, 1024, 1024, 16
NCORES = 8
HPC = H // NCORES            # heads per core = 2
HD = A // H                  # head dim = 64
E = HPC * HD                 # local qkv rows = 128
ND = D // 128                # d tiles = 8
NT = S // 128                # t tiles = 16
LN2 = 0.6931471805599453
EXP_SCALE = LN2 * (HD ** -0.5)   # p = 2^(score/8) = exp(score * ln2/8)

NCH = 2                      # attention s-chunks
CH = S // NCH                # 1024
NB = CH // 128               # proj s-blocks per chunk = 8

_CACHE = {}


def _build(enable_asserts=False):
    import concourse.bass as bass
    import concourse.tile as tile
    import concourse.mybir as mybir
    from concourse import bacc
    from concourse.masks import make_identity

    f16 = mybir.dt.float16

    nc = bacc.Bacc(
        "TRN2",
        target_bir_lowering=False,
        debug=False,
        enable_asserts=enable_asserts,
        num_devices=NCORES,
    )

    # xT: x.T as [ND, 128, S] (d-tile major); wqkv: [ND, 128, 3E] packed
    # q|k|v columns; wol: this core's w_out.T row strip [128, D].
    xT = nc.dram_tensor("xT", [ND, 128, S], f16, kind="ExternalInput").ap()
    wqkv = nc.dram_tensor("wqkv", [ND, 128, 3 * E], f16, kind="ExternalInput").ap()
    wol = nc.dram_tensor("wol", [128, D], f16, kind="ExternalInput").ap()
    # partial output, s-block major: rows s = ci*CH + b*128 + p
    out = nc.dram_tensor("out", [NCH, NB, 128, D], f16, kind="ExternalOutput").ap()

    with tile.TileContext(nc) as tc:
        _body(tc, xT, wqkv, wol, out, mybir, bass, make_identity)

    nc.compile()
    return nc


def _body(tc, xT, wqkv, wol, out, mybir, bass, make_identity):
    from contextlib import ExitStack

    nc = tc.nc
    f16 = mybir.dt.float16
    f32 = mybir.dt.float32
    Exp = mybir.ActivationFunctionType.Exp

    ctx_stack = ExitStack()
    persist = ctx_stack.enter_context(tc.tile_pool(name="persist", bufs=1))

    def ptile(shape, dtype, name):
        return persist.tile(shape, dtype, tag=name, name=name)

    # x.T d-tile major, split into one tile per load DMA for exact deps
    xt_g = [ptile([128, S], f16, f"xt_g{g}") for g in range(ND)]
    wqkv_sb = ptile([128, ND, 3 * E], f16, "wqkv_sb")
    wol_sb = ptile([128, D], f16, "wol_sb")
    # q.T split per attention chunk (exact deps on the q-pass copies)
    qT_c = [ptile([128, CH], f16, f"qT_c{ci}") for ci in range(NCH)]
    # per-head k.T zero-padded to K=128 so the scores matmul uses the same
    # (128,128) PE tile config as every other matmul
    kT2_sb = [ptile([128, S], f16, f"kT2_sb{h}") for h in range(HPC)]
    vT_sb = ptile([128, S], f16, "vT_sb")
    # v' per t-tile: [v_h0 | ones | v_h1 | ones] -> lhsT cols [0:65], [65:130]
    vp_sb = ptile([128, NT, 2 * (HD + 1)], f16, "vp_sb")
    ident_sb = ptile([128, 128], f16, "ident_sb")
    # normalized ctx.T, both heads: rows [h*64:(h+1)*64], cols = s
    ctxn_sb = ptile([128, S], f16, "ctxn_sb")
    # junk operand for PE clock-warmup matmuls (contents irrelevant)
    junk_sb = ptile([128, 512], f16, "junk_sb")

    make_identity(nc, ident_sb[:])

    # ---- batched input loads; the two HWDGE queues (sync, scalar)
    # drain in issue order and feed the DMA engines in parallel ----
    nc.sync.dma_start(wqkv_sb[:], wqkv.rearrange("t p c -> p t c"))
    nc.scalar.dma_start(wol_sb[:], wol)
    for g in range(ND):
        eng = nc.sync if g % 2 == 0 else nc.scalar
        eng.dma_start(xt_g[g][:], xT[g])

    # zero the pad halves of the per-head k tensors once, before the k-pass
    nc.vector.memset(kT2_sb[0][HD:128, :], 0.0)
    nc.vector.memset(kT2_sb[1][0:HD, :], 0.0)

    # PE clock warmup: dummy matmuls with no data deps run while the input
    # DMAs are still in flight, ramping the PE to the fast p-state before
    # the first qkv matmul
    nc.vector.memset(junk_sb[:], 0.0)
    with tc.tile_pool(name="warm_ps", bufs=1, space="PSUM") as warm_ps:
        wp = warm_ps.tile([128, 512], f32, tag="wp", name="warm")
        for i in range(16):
            nc.tensor.matmul(
                wp[:], lhsT=junk_sb[:, 0:128], rhs=junk_sb[:],
                start=True, stop=True,
            )

    # ---- qkv.T = w.T^T @ x.T : d-tile outer so each weight LDW feeds 4 MMs
    # pass order k, q, v (scores need k/q first). PSUM->SBUF copies on the
    # scalar engine (idle until the first Exp).
    with tc.tile_pool(name="qkv_ps", bufs=2, space="PSUM") as qkv_ps:
        for w_off, dst in ((E, None), (0, qT_c), (2 * E, vT_sb)):
            pss = [
                qkv_ps.tile([128, 512], f32, tag=f"qkv{i}", name=f"qkv{i}")
                for i in range(4)
            ]
            for dt_ in range(ND):
                for sc4 in range(4):
                    nc.tensor.matmul(
                        pss[sc4][:],
                        lhsT=wqkv_sb[:, dt_, w_off:w_off + E],
                        rhs=xt_g[dt_][:, sc4 * 512:(sc4 + 1) * 512],
                        start=(dt_ == 0),
                        stop=(dt_ == ND - 1),
                    )
            for sc4 in range(4):
                cols = slice(sc4 * 512, (sc4 + 1) * 512)
                if dst is None:      # k: split per head into zero-padded kT2
                    nc.scalar.copy(kT2_sb[0][0:HD, cols], pss[sc4][0:HD, :])
                    nc.scalar.copy(kT2_sb[1][HD:128, cols], pss[sc4][HD:128, :])
                elif isinstance(dst, list):   # q: per-chunk tiles
                    nc.scalar.copy(
                        dst[sc4 // 2][:, (sc4 % 2) * 512:(sc4 % 2 + 1) * 512],
                        pss[sc4][:],
                    )
                else:
                    nc.scalar.copy(dst[:, cols], pss[sc4][:])

    # ---- v' = v.T transposed back per t-tile, plus ones columns ----
    with tc.tile_pool(name="tr_ps", bufs=3, space="PSUM") as tr_ps:
        for tt in range(NT):
            tp = tr_ps.tile([128, 128], f16, tag="tr")
            nc.tensor.transpose(
                tp[:], vT_sb[:, tt * 128:(tt + 1) * 128], ident_sb[:]
            )
            nc.vector.tensor_copy(vp_sb[:, tt, 0:HD], tp[:, 0:HD])
            nc.vector.tensor_copy(
                vp_sb[:, tt, HD + 1:2 * HD + 1], tp[:, HD:2 * HD]
            )
        nc.vector.memset(vp_sb[:, :, HD:HD + 1], 1.0)
        nc.vector.memset(vp_sb[:, :, 2 * HD + 1:2 * HD + 2], 1.0)

    # ---- attention (chunk outer) + per-chunk partial projection ----
    with (
        tc.tile_pool(name="sc_ps", bufs=2, space="PSUM") as sc_ps,
        tc.tile_pool(name="ctx_ps", bufs=2, space="PSUM") as ctx_ps,
        tc.tile_pool(name="pt_pool", bufs=4) as pt_pool,
        tc.tile_pool(name="nrm_pool", bufs=2) as nrm_pool,
        tc.tile_pool(name="out_pool", bufs=2) as out_pool,
    ):
        def attn(ci, h, interleave=None):
            hb = h * HD
            ctx = ctx_ps.tile([HD + 1, CH], f32, tag="ctx", name="ctx")

            def ctx_mm(tt, pt):
                for nn in range(CH // 512):
                    nc.tensor.matmul(
                        ctx[:, nn * 512:(nn + 1) * 512],
                        lhsT=vp_sb[:, tt, h * (HD + 1):(h + 1) * (HD + 1)],
                        rhs=pt[:, nn * 512:(nn + 1) * 512],
                        start=(tt == 0),
                        stop=(tt == NT - 1),
                    )

            # software-pipelined: emit scores(tt) before ctx(tt-1) so the
            # in-order PE queue never stalls on exp(tt)
            pts = {}
            for tt in range(NT):
                sc = sc_ps.tile([128, CH], f32, tag="sc", name="sc")
                for nn in range(CH // 512):
                    nc.tensor.matmul(
                        sc[:, nn * 512:(nn + 1) * 512],
                        lhsT=kT2_sb[h][:, tt * 128:(tt + 1) * 128],
                        rhs=qT_c[ci][:, nn * 512:(nn + 1) * 512],
                        start=True,
                        stop=True,
                    )
                pt = pt_pool.tile([128, CH], f16, tag="pt")
                nc.scalar.activation(pt[:], sc[:], Exp, scale=EXP_SCALE)
                pts[tt] = pt
                if tt >= 1:
                    ctx_mm(tt - 1, pts.pop(tt - 1))
                if interleave is not None and tt in interleave:
                    interleave[tt]()
            ctx_mm(NT - 1, pts.pop(NT - 1))

            # normalize: copy+recip of the denominator row (the custom
            # DVE recip op cannot read PSUM), partition-broadcast, DVE
            # multiply into ctxn rows [hb:hb+64]. The last block runs in
            # column halves so the tail projection starts ~3us earlier.
            nhalves = 2 if (ci, h) == (NCH - 1, HPC - 1) else 1
            W = CH // nhalves
            for hf in range(nhalves):
                cs = slice(hf * W, (hf + 1) * W)
                den = nrm_pool.tile([1, W], f32, tag=f"den{nhalves}",
                                    name="den")
                nc.vector.tensor_copy(den[:], ctx[HD:HD + 1, cs])
                rec = nrm_pool.tile([1, W], f32, tag=f"rec{nhalves}",
                                    name="rec")
                nc.vector.reciprocal_approx_fast(rec[:], den[:])
                rb = nrm_pool.tile([HD, W], f32, tag=f"rb{nhalves}",
                                   name="rb")
                nc.gpsimd.partition_broadcast(rb[:], rec[:])
                nc.vector.tensor_mul(
                    ctxn_sb[hb:hb + HD,
                            ci * CH + hf * W:ci * CH + (hf + 1) * W],
                    ctx[0:HD, cs],
                    rb[:],
                )

        pair_obs = {}

        def proj_block(ci, b):
            # partial out rows [ci*CH + b*128 : +128]:
            # out[s, :] += ctxn[:, s].T @ wol  (K = this core's 128 A-rows)
            if b % 2 == 0:
                pair_obs[ci] = out_pool.tile([128, 2, D], f16, tag="ob",
                                             name="ob")
            ob = pair_obs[ci]
            ps = sc_ps.tile([128, CH], f32, tag="sc", name="proj")
            for nn in range(2):
                nc.tensor.matmul(
                    ps[:, nn * 512:(nn + 1) * 512],
                    lhsT=ctxn_sb[:, ci * CH + b * 128:
                                 ci * CH + (b + 1) * 128],
                    rhs=wol_sb[:, nn * 512:(nn + 1) * 512],
                    start=True,
                    stop=True,
                )
            nc.vector.tensor_copy(ob[:, b % 2, :], ps[:])
            if b % 2 == 1:   # ship each 2-block pair as soon as it's cast
                nc.sync.dma_start(
                    out[ci, b - 1:b + 1].rearrange("b p d -> p b d"), ob[:]
                )

        attn(0, 0)
        attn(0, 1)
        # proj(0) blocks are interleaved into attn(1,0)'s tt loop: the
        # scalar engine keeps running exps while the PE absorbs the
        # projection matmuls in its slack.
        attn(1, 0, interleave={7 + i: (lambda i=i: proj_block(0, i))
                               for i in range(NB)})
        attn(1, 1)
        # dummy matmuls on junk data bridge the normalize(1,1) window so
        # the PE clock stays at the fast p-state for the tail projection
        for i in range(10):
            jp = sc_ps.tile([128, CH], f32, tag="sc", name="junk")
            nc.tensor.matmul(
                jp[:, 0:512], lhsT=junk_sb[:, 0:128],
                rhs=junk_sb[:, 0:512], start=True, stop=True,
            )
        for b in range(NB):
            proj_block(1, b)

    ctx_stack.close()


def get_nc(enable_asserts=False):
    key = ("nc", enable_asserts)
    if key not in _CACHE:
        _CACHE[key] = _build(enable_asserts)
    return _CACHE[key]


def make_in_maps(x, w_in, w_out):
    x = np.asarray(x, dtype=np.float32)
    w_in = np.asarray(w_in, dtype=np.float32)
    w_out = np.asarray(w_out, dtype=np.float32)
    xT = np.ascontiguousarray(x.T).astype(np.float16).reshape(ND, 128, S)
    w_outT = np.ascontiguousarray(w_out.T).astype(np.float16)  # [A, D]
    in_maps = []
    for c in range(NCORES):
        r0 = c * E
        wq = w_in[r0:r0 + E].T
        wk = w_in[A + r0:A + r0 + E].T
        wv = w_in[2 * A + r0:2 * A + r0 + E].T
        wqkv = np.ascontiguousarray(
            np.concatenate([wq, wk, wv], axis=1)
        ).astype(np.float16).reshape(ND, 128, 3 * E)
        wol = np.ascontiguousarray(w_outT[r0:r0 + E])  # [128, D]
        in_maps.append({"xT": xT, "wqkv": wqkv, "wol": wol})
    return in_maps


def assemble_out(results):
    """results[c]["out"] is [NCH, NB, 128, D] fp16 partials in s-block
    order; the unshard step sums the 8 cores' partial projections."""
    full = np.zeros((S, D), dtype=np.float32)
    for c in range(NCORES):
        o = results[c]["out"].astype(np.float32).reshape(S, D)
        full += o
    return full


def kernel(x, w_in, w_out, tgt_len=None, **kwargs):
    from concourse.bass_utils import run_bass_kernel_spmd

    nc = get_nc()
    in_maps = make_in_maps(x, w_in, w_out)
    res = run_bass_kernel_spmd(nc, in_maps, core_ids=list(range(NCORES)))
    return assemble_out(res.results)
